# revision 1
# baseline (speedup 1.0000x reference)
"""Trainium2 Bass kernel for nn_ELM_AE_FatSpectral_Ensemble.

Data-parallel over batch: 4 samples/core on 8 cores. v2 design:
  - Input stays channel-major [c, sp*sp]; NO PE transpose of the 24MB input.
  - Bilinear+antialias resize applied as a separable 2-pass FIR
    (row pass then col pass) on vector/gpsimd/scalar engines using
    strided access patterns (taps extracted from the resize matrix R:
    rows 1..12 share one shifted tap vector; rows 0/13 are edge cases).
  - Only the resized Xr [c, 196] (12MB/core) is transposed on the PE
    to pixel-major XrT for the pixel-contraction matmuls.
  - zscore is FOLDED into the matmuls (never materialized):
      H = sigmoid(sc .* (W@Xr + sW (x) (-mu)))   [channel-major matmul]
      C^T = (sc.*H^T)^T @ XrT - d (x) 1,  d = (sc.*H^T)^T @ mu
  - G = H H^T, G^-1 via Newton-Schulz on block-diag [128,128]
    supermatrices (2 supers of 2 samples x 4 members), variance readout
    as in: out_i = quad_i/(Q-1) - t_i^2/(Q(Q-1)).
"""

import numpy as np

import concourse.bacc as bacc
import concourse.tile as tile
from concourse import mybir
from concourse.bass_utils import run_bass_kernel_spmd

F32 = mybir.dt.float32
F32R = mybir.dt.float32r
AF = mybir.ActivationFunctionType
ALU = mybir.AluOpType

S = 4
NCORES = 8
SP = 14
WH = SP * SP
Q = 16
MEMBERS = [(256, 56), (512, 28), (1024, 14), (2048, 7)]
OFFS = [0, 256, 768, 1792]
DTOT = 3840
NEWTON_ITERS = 10


def _weight_mat(n_in, n_out):
    scale = n_out / n_in
    kernel_scale = max(1.0, 1.0 / scale)
    sample_f = (np.arange(n_out) + 0.5) / scale - 0.5
    x = np.abs(sample_f[:, None] - np.arange(n_in)[None, :]) / kernel_scale
    w = np.maximum(0.0, 1.0 - x)
    total = w.sum(axis=1, keepdims=True)
    return (w / np.where(total > 0, total, 1)).astype(np.float32)


def _taps():
    """Per-member separable tap schedule, verified against _weight_mat.

    Returns {m: (kind, main, edge0, edge13)} where main rows i=1..12 use
    out[i] += w[j] * in[st*i + j + off] and edges list (u, w) pairs.
    m=3 (7->14 upsample) is handled specially (kind='up').
    """
    taps = {}
    for m, (c, sp) in enumerate(MEMBERS):
        if sp == SP:
            continue
        R = _weight_mat(sp, SP)
        if sp > SP:
            st = sp // SP
            nt = 2 * st
            off = -(st // 2)
            w = R[6, st * 6 + off:st * 6 + off + nt].copy()
            for i in range(1, 13):
                assert np.allclose(R[i, st * i + off:st * i + off + nt], w)
            e0 = [(u, R[0, u]) for u in range(sp) if R[0, u] != 0]
            e13 = [(u, R[13, u]) for u in range(sp) if R[13, u] != 0]
            assert np.allclose(w, w[::-1])
            taps[m] = ("down", st, off, w, e0, e13)
        else:
            assert sp == 7
            assert R[0, 0] == 1.0 and R[13, 6] == 1.0
            assert np.allclose(R[1, 0:2], [0.75, 0.25])
            assert np.allclose(R[2, 0:2], [0.25, 0.75])
            taps[m] = ("up", None, None, None, None, None)
    return taps


def _consts():
    ident = np.eye(128, dtype=np.float32)
    p16 = np.kron(np.eye(8, dtype=np.float32), np.ones((16, 16), np.float32))
    p16 = p16.astype(np.float32)
    mask8 = np.zeros((4, 2, 128, 4), np.float32)
    for m in range(4):
        for spr in range(2):
            for si in range(2):
                b = 4 * si + m
                mask8[m, spr, 16 * b:16 * b + 16, 2 * spr + si] = 1.0
    mask8 = np.ascontiguousarray(mask8.transpose(2, 0, 1, 3))  # [128, 4, 2, 4]
    R7 = _weight_mat(7, SP)
    k3 = np.ascontiguousarray(np.kron(R7, R7).T.astype(np.float32))  # [49,196]
    return ident, p16, mask8, k3


def _chunks(n, sz=128):
    return [(i, min(sz, n - i)) for i in range(0, n, sz)]


# DMA granularity: c-chunks per input DMA tile.  m0/m1 use 1 so every
# resize AP stays <=3 dims (BIR verifier limit); m2/m3 have no vector
# resize so they batch the whole sample per DMA.
GSZ = {0: 1, 1: 1, 2: 8, 3: 16}
# xr granule: c-chunks per resized tile (m0/m1 only)
XRG = {0: 1, 1: 1}


def _build_program():
    ident_np, p16_np, mask8_np, k3_np = _consts()
    taps = _taps()

    nc = bacc.Bacc()
    xin, wt = {}, {}
    for m, (c, sp) in enumerate(MEMBERS):
        xin[m] = nc.dram_tensor(f"x{m}", [S, c, sp * sp], F32R,
                                kind="ExternalInput")
        wt[m] = nc.dram_tensor(f"wt{m}", [c, Q], F32R, kind="ExternalInput")
    identd = nc.dram_tensor("ident", [128, 128], F32, kind="ExternalInput")
    p16d = nc.dram_tensor("p16", [128, 128], F32, kind="ExternalInput")
    mask8d = nc.dram_tensor("mask8", [128, 4, 2, 4], F32, kind="ExternalInput")
    k3d = nc.dram_tensor("k3", [49, WH], F32, kind="ExternalInput")
    onesd = nc.dram_tensor("onesd", [128, Q], F32, kind="ExternalInput")
    outd = nc.dram_tensor("out", [S, DTOT], F32, kind="ExternalOutput")

    from contextlib import ExitStack
    _ceng = [0]
    _deng = [0]

    def _pcopy(out, in_):
        # PSUM -> SBUF copies rotate scalar/vector (gpsimd cannot read PSUM)
        _ceng[0] ^= 1
        if _ceng[0]:
            nc.scalar.copy(out=out, in_=in_)
        else:
            nc.vector.tensor_copy(out=out, in_=in_)

    def _dma(out, in_):
        _deng[0] ^= 1
        (nc.sync if _deng[0] else nc.gpsimd).dma_start(out=out, in_=in_)

    def _xtcopy(dst, src, acc):
        del acc
        _pcopy(out=dst, in_=src)

    def _stt(out, in0, w, in1):
        nc.vector.scalar_tensor_tensor(out=out, in0=in0, scalar=float(w),
                                       in1=in1, op0=ALU.mult, op1=ALU.add)

    def _scale(out, in_, w):
        nc.vector.tensor_scalar_mul(out, in_, float(w))

    def _escale(out, in_, w):
        # edge-row first tap on the scalar engine
        nc.scalar.activation(out=out, in_=in_, func=AF.Copy, scale=float(w))

    with tile.TileContext(nc) as tc, ExitStack() as _es:
        _es.enter_context(
            nc.allow_low_precision(reason="f32r-rounded PE operands"))
        _p = lambda **kw: _es.enter_context(tc.tile_pool(**kw))
        consts = _p(name="consts", bufs=1)
        xinp = _p(name="xinp", bufs=2)
        yp = _p(name="yp", bufs=2)
        xrp = _p(name="xrp", bufs=2)
        xtp = _p(name="xtp", bufs=1)
        hp = _p(name="hp", bufs=2)
        htp = _p(name="htp", bufs=2)
        sup = _p(name="sup", bufs=1)
        newt = _p(name="newt", bufs=2)
        smalls = _p(name="smalls", bufs=4)
        rowsp = _p(name="rowsp", bufs=2)
        psbp = _p(name="psbp", bufs=2)
        tsqp = _p(name="tsqp", bufs=1)
        outp = _p(name="outp", bufs=2)
        pt = _p(name="pt", bufs=2, space="PSUM")
        pacc = _p(name="pacc", bufs=2, space="PSUM")
        hq = _p(name="hq", bufs=2, space="PSUM")

        # ---------- constants ----------
        ident_sb = consts.tile([128, 128], F32, tag="ident")
        nc.gpsimd.dma_start(out=ident_sb, in_=identd[:, :])
        identr_sb = consts.tile([128, 128], F32R, tag="identr")
        nc.gpsimd.dma_start(out=identr_sb, in_=identd[:, :].bitcast(F32R))
        p16_sb = consts.tile([128, 128], F32, tag="p16")
        nc.gpsimd.dma_start(out=p16_sb, in_=p16d[:, :])
        mask8_sb = consts.tile([128, 4, 2, 4], F32, tag="mask8")
        nc.gpsimd.dma_start(out=mask8_sb, in_=mask8d[:, :, :, :])
        oq8_sb = consts.tile([128, 4, 2, 4], F32, tag="oq8")
        nc.vector.tensor_scalar_mul(
            oq8_sb.rearrange("p a b c -> p (a b c)").bitcast(F32R),
            mask8_sb.rearrange("p a b c -> p (a b c)"), 1.0 / (Q - 1))
        ones_sb = consts.tile([128, 1], F32, tag="ones")
        nc.vector.memset(ones_sb, 1.0)
        onesr_sb = consts.tile([128, Q], F32R, tag="onesr")
        nc.gpsimd.dma_start(out=onesr_sb, in_=onesd[:, :].bitcast(F32R))
        zer_sb = consts.tile([128, 256], F32, tag="zer")
        nc.vector.memset(zer_sb, 0.0)
        k3_sb = consts.tile([49, WH], F32R, tag="k3")
        nc.gpsimd.dma_start(out=k3_sb, in_=k3d[:, :].bitcast(F32R))

        wtt = {}
        for m, (c, sp) in enumerate(MEMBERS):
            cc_n = c // 128
            wtt[m] = consts.tile([128, cc_n, Q], F32R, tag=f"wt{m}",
                                 name=f"wt{m}")
            nc.gpsimd.dma_start(
                out=wtt[m],
                in_=wt[m][:, :].rearrange("(k p) q -> p k q", p=128))

        # sW rows: sw[m] = sum_c W[q,c] as a [1, Q] row
        swrow = {}
        for m, (c, sp) in enumerate(MEMBERS):
            cc_n = c // 128
            swp = pacc.tile([128, 512], F32, tag="pacc")
            for cc in range(cc_n):
                nc.tensor.matmul(swp[:1, 0:Q], lhsT=onesr_sb[:, 0:1],
                                 rhs=wtt[m][:, cc, :],
                                 start=(cc == 0), stop=(cc == cc_n - 1))
            swrow[m] = consts.tile([1, Q], F32, tag=f"sw{m}", name=f"sw{m}")
            nc.vector.tensor_copy(out=swrow[m].bitcast(F32R),
                                  in_=swp[:1, 0:Q])

        ct_all, gsup = [], []
        for spr in range(2):
            t = sup.tile([128, DTOT], F32, tag=f"ct{spr}", name=f"ct{spr}")
            ct_all.append(t)
            g = sup.tile([128, 128], F32, tag=f"gsup{spr}", name=f"gsup{spr}")
            nc.vector.memset(g, 0.0)
            gsup.append(g)

        m2_sb = [None, None]
        r_sb = [None, None]

        # ============ phase 1+2, super-outer =============
        for spr in range(2):
            for m, (c, sp) in enumerate(MEMBERS):
                uv = sp * sp
                cc_n = c // 128
                gsz = GSZ[m]
                xrg = XRG.get(m, 1)
                nch = _chunks(c, 512)

                # ---- XrT (pixel-major) destination tiles ----
                x0t, x1t = {}, {}
                par = m % 2
                for si in range(2):
                    x0t[si] = xtp.tile([128, c], F32, tag=f"x0t_{si}_{par}",
                                       name=f"x0t_{si}")
                    x1t[si] = xtp.tile([68, c], F32, tag=f"x1t_{si}_{par}",
                                       name=f"x1t_{si}")
                acs = smalls.tile([128, 2, 2, 8], F32, tag="acs")
                acq = smalls.tile([128, 2, 2, 4], F32, tag="acq")

                # hps PSUM accumulator for H = W @ Xr (pair-batched)
                hps = hq.tile([Q, 2 * WH], F32, tag="hq")
                hmm = [0]  # H matmul counter (for start flag)

                def _h_mm(rhs_ap):
                    nc.tensor.matmul(hps, lhsT=wtt[m][:, hmm[0], :],
                                     rhs=rhs_ap,
                                     start=(hmm[0] == 0), stop=False)
                    hmm[0] += 1

                tp_st = {0: None, 1: None}

                def _transpose_chunk(src_ap, cc_abs, si):
                    # src_ap: [128, 196] channel-major chunk -> XrT cols.
                    # Chunks are paired into one [128,512] PSUM tile so the
                    # PSUM->SBUF copies are 256 wide (and carry sum-accum).
                    if tp_st[si] is None:
                        ps = pt.tile([128, 512], F32R, tag="pt")
                        nc.tensor.transpose(ps[:128, 0:128],
                                            src_ap[:, 0:128].bitcast(F32R),
                                            identr_sb)
                        nc.tensor.transpose(ps[:68, 256:384],
                                            src_ap[:, 128:196].bitcast(F32R),
                                            identr_sb)
                        tp_st[si] = (ps, cc_abs)
                        return
                    ps, cc0 = tp_st[si]
                    assert cc_abs == cc0 + 1
                    nc.tensor.transpose(ps[:128, 128:256],
                                        src_ap[:, 0:128].bitcast(F32R),
                                        identr_sb)
                    nc.tensor.transpose(ps[:68, 384:512],
                                        src_ap[:, 128:196].bitcast(F32R),
                                        identr_sb)
                    co = cc0 * 128
                    slot = cc0 // 2
                    _xtcopy(dst=x0t[si][:, co:co + 256].bitcast(F32R),
                            src=ps[:128, 0:256],
                            acc=acs[:128, si, 0, slot:slot + 1])
                    _xtcopy(dst=x1t[si][:68, co:co + 256].bitcast(F32R),
                            src=ps[:68, 256:512],
                            acc=acs[:68, si, 1, slot:slot + 1])
                    tp_st[si] = None

                if m == 2:
                    # identity spatial size: xin IS Xr; interleave si pairs
                    for g0 in range(0, cc_n, gsz):
                        gn = min(gsz, cc_n - g0)
                        xt_in = xinp.tile([128, gsz, 2, WH], F32R,
                                          tag="xin_0", name="xt_in2")
                        for si in range(2):
                            s = 2 * spr + si
                            _dma(
                                out=xt_in[:, :gn, si, :],
                                in_=xin[m][s, g0 * 128:(g0 + gn) * 128,
                                           :].rearrange(
                                    "(k p) v -> p k v", p=128))
                        for g in range(gn):
                            _h_mm(xt_in[:, g, :, :].rearrange(
                                "p a b -> p (a b)").bitcast(F32R))
                            for si in range(2):
                                _transpose_chunk(
                                    xt_in[:, g, si, :], g0 + g, si)
                elif m == 3:
                    # 7x7 input is tiny: transpose RAW x3 on the PE, resize
                    # via matmul against kron(R7,R7).T, and build H^T on the
                    # pixel side (wx = W@X3, resized through k3 transposed).
                    x3t = xtp.tile([49, 2, c], F32, tag="x3t", name="x3t")
                    wxp = hq.tile([49, 2, Q], F32, tag="hq")
                    wxt = smalls.tile([49, 2, Q], F32, tag="wxt")
                    for si in range(2):
                        s = 2 * spr + si
                        xt_in = xinp.tile([128, gsz, uv], F32R,
                                          tag=f"xin_{si}", name=f"xt_in{si}")
                        _dma(out=xt_in[:, :, :],
                             in_=xin[m][s, :, :].rearrange(
                                 "(k p) v -> p k v", p=128))
                        for g in range(cc_n):
                            # wx^T accumulation [49, 16] per si
                            nc.tensor.matmul(
                                wxp[:uv, si, :],
                                lhsT=xt_in[:, g, :].bitcast(F32R),
                                rhs=wtt[m][:, g, :],
                                start=(g == 0), stop=(g == cc_n - 1))
                            ps = pt.tile([128, 256], F32R, tag="pt")
                            nc.tensor.transpose(
                                ps[:uv, 0:128],
                                xt_in[:, g, :].bitcast(F32R), identr_sb)
                            _pcopy(out=x3t[:uv, si,
                                           g * 128:(g + 1) * 128].bitcast(
                                               F32R),
                                   in_=ps[:uv, 0:128])
                        _pcopy(out=wxt[:uv, si, :].bitcast(F32R),
                               in_=wxp[:uv, si, :])
                    # resize matmuls -> XrT tiles
                    for si in range(2):
                        for ni, (no, nsz) in enumerate(nch):
                            rp = pacc.tile([128, 512], F32, tag="pacc")
                            nc.tensor.matmul(
                                rp[:128, :nsz], lhsT=k3_sb[:uv, 0:128],
                                rhs=x3t[:uv, si, no:no + nsz].bitcast(F32R),
                                start=True, stop=True)
                            _xtcopy(dst=x0t[si][:, no:no +
                                                    nsz].bitcast(F32R),
                                    src=rp[:128, :nsz],
                                    acc=acs[:128, si, 0, ni:ni + 1])
                            rp2 = pacc.tile([128, 512], F32, tag="pacc")
                            nc.tensor.matmul(
                                rp2[:68, :nsz], lhsT=k3_sb[:uv, 128:196],
                                rhs=x3t[:uv, si, no:no + nsz].bitcast(F32R),
                                start=True, stop=True)
                            _xtcopy(dst=x1t[si][:68, no:no +
                                                    nsz].bitcast(F32R),
                                    src=rp2[:68, :nsz],
                                    acc=acs[:68, si, 1, ni:ni + 1])
                else:
                    kind, st, off, wmain, e0, e13 = taps[m]
                    for g0 in range(0, cc_n, gsz):
                        gn = min(gsz, cc_n - g0)
                        yts = {}
                        for si in range(2):
                            s = 2 * spr + si
                            xt_in = xinp.tile([128, gsz, uv], F32R,
                                              tag=f"xin_{si}",
                                              name=f"xt_in{si}")
                            _dma(
                                out=xt_in[:, :gn, :],
                                in_=xin[m][s, g0 * 128:(g0 + gn) * 128,
                                           :].rearrange(
                                    "(k p) v -> p k v", p=128))
                            x4 = xt_in.rearrange(
                                "p k (u v) -> p k u v",
                                v=sp)[:, :gn, :, :].bitcast(F32)
                            # ---- row pass: Y[128, gn, 14, sp] ----
                            yt = yp.tile([128, gsz, SP, sp], F32,
                                         tag=f"y_{si}", name=f"y{si}")
                            y4 = yt[:, :gn, :, :]
                            for j, w in enumerate(wmain):
                                u0 = st + off + j
                                src_ = x4[:, 0, u0:u0 + 11 * st + 1:st, :]
                                if j == 0:
                                    _scale(y4[:, 0, 1:13, :], src_, w)
                                else:
                                    _stt(y4[:, 0, 1:13, :], src_, w,
                                         y4[:, 0, 1:13, :])
                            for j, (u, w) in enumerate(e0):
                                if j == 0:
                                    _escale(y4[:, 0, 0, :],
                                            x4[:, 0, u, :], w)
                                else:
                                    _stt(y4[:, 0, 0, :], x4[:, 0, u, :],
                                         w, y4[:, 0, 0, :])
                            for j, (u, w) in enumerate(e13):
                                if j == 0:
                                    _escale(y4[:, 0, 13, :],
                                            x4[:, 0, u, :], w)
                                else:
                                    _stt(y4[:, 0, 13, :], x4[:, 0, u, :],
                                         w, y4[:, 0, 13, :])
                            yts[si] = yt
                        # ---- col pass + H + transposes per xr granule ----
                        for h0 in range(0, gn, xrg):
                            hn = min(xrg, gn - h0)
                            xrt = xrp.tile([128, xrg, 2, WH], F32,
                                           tag="xr", name="xr")
                            for si in range(2):
                                x5 = xrt[:, :hn, si, :].rearrange(
                                    "p g (i j) -> p g i j", j=SP)
                                yv = yts[si][:, h0:h0 + hn, :, :]
                                for j, w in enumerate(wmain):
                                    u0 = st + off + j
                                    src_ = yv[:, 0, :,
                                              u0:u0 + 11 * st + 1:st]
                                    if j == 0:
                                        _scale(x5[:, 0, :, 1:13].bitcast(
                                            F32R), src_, w)
                                    else:
                                        _stt(x5[:, 0, :, 1:13].bitcast(F32R),
                                             src_, w, x5[:, 0, :, 1:13])
                                for j, (u, w) in enumerate(e0):
                                    if j == 0:
                                        _escale(x5[:, 0, :, 0].bitcast(F32R),
                                                yv[:, 0, :, u], w)
                                    else:
                                        _stt(x5[:, 0, :, 0].bitcast(F32R),
                                             yv[:, 0, :, u], w,
                                             x5[:, 0, :, 0])
                                for j, (u, w) in enumerate(e13):
                                    if j == 0:
                                        _escale(x5[:, 0, :, 13].bitcast(
                                            F32R), yv[:, 0, :, u], w)
                                    else:
                                        _stt(x5[:, 0, :, 13].bitcast(F32R),
                                             yv[:, 0, :, u], w,
                                             x5[:, 0, :, 13])
                            for g in range(hn):
                                _h_mm(xrt[:, g, :, :].rearrange(
                                    "p a b -> p (a b)").bitcast(F32R))
                                for si in range(2):
                                    _transpose_chunk(
                                        xrt[:, g, si, :], g0 + h0 + g, si)

                # ---- zscore stats: sum + sumsq passes on XrT ----
                scq, muq, nmq = {}, {}, {}
                for si in range(2):
                    for wi, (tl, psz) in enumerate(((x0t[si], 128),
                                                    (x1t[si], 68))):
                        gs = _chunks(c, 512)
                        s1 = smalls.tile([128, 1], F32, tag="s1")
                        nc.vector.tensor_reduce(
                            out=s1[:psz], in_=tl[:psz, :],
                            axis=mybir.AxisListType.X, op=ALU.add)
                        for gi, (go, gln) in enumerate(gs):
                            scr = psbp.tile([128, 512], F32, tag="psb")
                            nc.scalar.square(out=scr[:psz, :gln],
                                             in_=tl[:psz, go:go + gln])
                            nc.vector.tensor_reduce(
                                out=acq[:psz, si, wi, gi:gi + 1],
                                in_=scr[:psz, :gln],
                                axis=mybir.AxisListType.X, op=ALU.add)
                        s2 = smalls.tile([128, 1], F32, tag="s2")
                        if len(gs) == 1:
                            s2 = None
                            s2ap = acq[:psz, si, wi, 0:1]
                        else:
                            nc.vector.tensor_reduce(
                                out=s2[:psz],
                                in_=acq[:psz, si, wi, 0:len(gs)],
                                axis=mybir.AxisListType.X, op=ALU.add)
                            s2ap = s2[:psz]
                        mean = smalls.tile([128, 1], F32, tag="mean")
                        nc.vector.tensor_scalar_mul(mean[:psz], s1[:psz],
                                                    1.0 / c)
                        uvar = smalls.tile([128, 1], F32, tag="uvar")
                        nc.vector.tensor_scalar(
                            out=uvar[:psz], in0=s1[:psz],
                            scalar1=mean[:psz], scalar2=None, op0=ALU.mult)
                        nc.vector.tensor_tensor(
                            out=uvar[:psz], in0=s2ap, in1=uvar[:psz],
                            op=ALU.subtract)
                        sd = smalls.tile([128, 1], F32, tag="sd")
                        nc.scalar.activation(out=sd[:psz], in_=uvar[:psz],
                                             func=AF.Sqrt,
                                             scale=1.0 / (c - 1.0))
                        sc = smalls.tile([128, 1], F32, tag=f"sc{si}{wi}",
                                         name=f"sc{si}{wi}")
                        nc.vector.reciprocal(out=sc[:psz].bitcast(F32R),
                                             in_=sd[:psz])
                        mu = smalls.tile([128, 2], F32, tag=f"mu{si}{wi}",
                                         name=f"mu{si}{wi}")
                        nc.vector.tensor_copy(
                            out=mu[:psz, 0:1].bitcast(F32R), in_=mean[:psz])
                        nc.vector.tensor_copy(
                            out=mu[:psz, 1:2].bitcast(F32R), in_=mean[:psz])
                        nm = smalls.tile([128, 1], F32, tag=f"nm{si}{wi}",
                                         name=f"nm{si}{wi}")
                        nc.vector.tensor_scalar_mul(
                            nm[:psz].bitcast(F32R), mean[:psz], -1.0)
                        scq[(si, wi)] = (sc, psz)
                        muq[(si, wi)] = (mu, psz)
                        nmq[(si, wi)] = (nm, psz)

                # rows: scrow/nmrow [1, 2, 196]
                scrow = rowsp.tile([1, 2, WH], F32, tag="scrow")
                nmrow = rowsp.tile([1, 2, WH], F32, tag="nmrow")
                for si in range(2):
                    for wi, (o, psz) in ((0, (0, 128)), (1, (128, 68))):
                        sc, _ = scq[(si, wi)]
                        nm, _ = nmq[(si, wi)]
                        pr = pt.tile([128, 256], F32R, tag="pt")
                        nc.tensor.transpose(pr[:1, 0:psz],
                                            sc[:psz, 0:1].bitcast(F32R),
                                            identr_sb[:psz, :psz])
                        nc.tensor.transpose(pr[:1, 128:128 + psz],
                                            nm[:psz, 0:1].bitcast(F32R),
                                            identr_sb[:psz, :psz])
                        _pcopy(out=scrow[0:1, si, o:o + psz].bitcast(F32R),
                               in_=pr[:1, 0:psz])
                        _pcopy(out=nmrow[0:1, si, o:o + psz].bitcast(F32R),
                               in_=pr[:1, 128:128 + psz])

                if m == 3:
                    pass
                else:
                    # rank-1 zscore term closes the hps accumulation
                    nc.tensor.matmul(
                        hps, lhsT=swrow[m].bitcast(F32R),
                        rhs=nmrow.rearrange("p a b -> p (a b)").bitcast(F32R),
                        start=False, stop=True)
                    # souter = ones16 (x) scrow
                    sop = pacc.tile([128, 512], F32, tag="pacc")
                    nc.tensor.matmul(
                        sop[:Q, 0:2 * WH], lhsT=onesr_sb[0:1, :],
                        rhs=scrow.rearrange("p a b -> p (a b)").bitcast(F32R),
                        start=True, stop=True)
                    sout = hp.tile([Q, 2 * WH], F32, tag="sout")
                    _pcopy(out=sout, in_=sop[:Q, 0:2 * WH])
                    tmph = hp.tile([Q, 2 * WH], F32, tag="tmph")
                    nc.vector.tensor_mul(tmph, hps, sout)
                    hsb = hp.tile([Q, 2, WH], F32, tag="hsb")
                    nc.scalar.activation(
                        out=hsb.rearrange("p a b -> p (a b)").bitcast(F32R),
                        in_=tmph, func=AF.Sigmoid)

                # ---- H^T supermatrix blocks, scaled variant, G ----
                htbig, htsb = {}, {}
                gp = pacc.tile([128, 512], F32, tag="pacc")
                for si in range(2):
                    b = 4 * si + m
                    bo = 16 * b
                    htb = htp.tile([128, 2, 128], F32, tag=f"htbig{si}",
                                   name=f"htbig{si}")
                    nc.gpsimd.tensor_copy(
                        out=htb.rearrange("p a b -> p (a b)").bitcast(F32R),
                        in_=zer_sb)
                    if m == 3:
                        php = pacc.tile([128, 512], F32, tag="pacc")
                        nc.tensor.matmul(
                            php[:128, 0:Q], lhsT=k3_sb[:uv, 0:128],
                            rhs=wxt[:uv, si, :].bitcast(F32R),
                            start=True, stop=False)
                        nc.tensor.matmul(
                            php[:128, 0:Q],
                            lhsT=nmrow[0:1, si, 0:128].bitcast(F32R),
                            rhs=swrow[m].bitcast(F32R),
                            start=False, stop=True)
                        php2 = pacc.tile([128, 512], F32, tag="pacc")
                        nc.tensor.matmul(
                            php2[:68, 0:Q], lhsT=k3_sb[:uv, 128:196],
                            rhs=wxt[:uv, si, :].bitcast(F32R),
                            start=True, stop=False)
                        nc.tensor.matmul(
                            php2[:68, 0:Q],
                            lhsT=nmrow[0:1, si, 128:196].bitcast(F32R),
                            rhs=swrow[m].bitcast(F32R),
                            start=False, stop=True)
                        nc.scalar.activation(
                            out=htb[:128, 0, bo:bo + Q].bitcast(F32R),
                            in_=php[:128, 0:Q], func=AF.Sigmoid,
                            scale=scq[(si, 0)][0][:128])
                        nc.scalar.activation(
                            out=htb[:68, 1, bo:bo + Q].bitcast(F32R),
                            in_=php2[:68, 0:Q], func=AF.Sigmoid,
                            scale=scq[(si, 1)][0][:68])
                    else:
                        pht = pt.tile([128, 256], F32R, tag="pt")
                        nc.tensor.transpose(pht[:128, 0:Q],
                                            hsb[:Q, si, 0:128].bitcast(F32R),
                                            identr_sb[:Q, :Q])
                        nc.tensor.transpose(pht[:68, Q:2 * Q],
                                            hsb[:Q, si,
                                                128:196].bitcast(F32R),
                                            identr_sb[:Q, :Q])
                        nc.vector.tensor_copy(
                            out=htb[:, 0, bo:bo + Q].bitcast(F32R),
                            in_=pht[:128, 0:Q])
                        nc.vector.tensor_copy(
                            out=htb[:68, 1, bo:bo + Q].bitcast(F32R),
                            in_=pht[:68, Q:2 * Q])
                    htbig[si] = htb
                    hts = htp.tile([128, 2, 128], F32, tag=f"hts{si}",
                                   name=f"hts{si}")
                    for wi, psz in ((0, 128), (1, 68)):
                        sc, _ = scq[(si, wi)]
                        nc.vector.tensor_scalar(
                            out=hts[:psz, wi, :].bitcast(F32R),
                            in0=htb[:psz, wi, :], scalar1=sc[:psz],
                            scalar2=None, op0=ALU.mult)
                    htsb[si] = hts
                    for wi, wsz in ((0, 128), (1, 68)):
                        nc.tensor.matmul(gp[:, :128],
                                         lhsT=htb[:wsz, wi, :],
                                         rhs=htb[:wsz, wi, :],
                                         start=(si == 0 and wi == 0),
                                         stop=(si == 1 and wi == 1))
                nc.vector.tensor_tensor(out=gsup[spr], in0=gsup[spr],
                                        in1=gp[:, :128], op=ALU.add)

                # ---- d = hts^T @ mu, then C^T chunks with -d fixup ----
                dps = pacc.tile([128, 512], F32, tag="pacc")
                idx = 0
                for si in range(2):
                    for wi, wsz in ((0, 128), (1, 68)):
                        mu, _ = muq[(si, wi)]
                        nc.tensor.matmul(
                            dps[:, 0:2],
                            lhsT=htsb[si][:wsz, wi, :].bitcast(F32R),
                            rhs=mu[:wsz, 0:2].bitcast(F32R),
                            start=(idx == 0), stop=(idx == 3))
                        idx += 1
                negd = smalls.tile([128, 1], F32, tag="negd")
                nc.vector.tensor_scalar_mul(negd, dps[:, 0:1], -1.0)

                for ni, (no, nsz) in enumerate(nch):
                    cps = pacc.tile([128, 512], F32, tag="pacc")
                    idx = 0
                    for si in range(2):
                        for wi, wsz, xs in ((0, 128, x0t[si]),
                                            (1, 68, x1t[si])):
                            nc.tensor.matmul(
                                cps[:, :nsz],
                                lhsT=htsb[si][:wsz, wi, :].bitcast(F32R),
                                rhs=xs[:wsz, no:no + nsz].bitcast(F32R),
                                start=(idx == 0), stop=(idx == 3))
                            idx += 1
                    dst = ct_all[spr][:, OFFS[m] + no:OFFS[m] + no +
                                      nsz].bitcast(F32R)
                    if ni % 2:
                        nc.vector.tensor_scalar(
                            out=dst, in0=cps[:, :nsz], scalar1=negd,
                            scalar2=None, op0=ALU.add)
                    else:
                        nc.scalar.activation(out=dst, in_=cps[:, :nsz],
                                             func=AF.Identity, bias=negd)

            # ---- phase 2 for this super: Newton-Schulz ----
            g = gsup[spr]
            sq = newt.tile([128, 128], F32, tag="sq")
            nc.vector.tensor_mul(sq, g, g)
            rs = newt.tile([128, 1], F32, tag="rs")
            nc.vector.tensor_reduce(out=rs, in_=sq,
                                    axis=mybir.AxisListType.X, op=ALU.add)
            bps = pacc.tile([128, 512], F32, tag="pacc")
            nc.tensor.matmul(bps[:128, 0:1], lhsT=p16_sb, rhs=rs,
                             start=True, stop=True)
            bf = newt.tile([128, 1], F32, tag="bf")
            nc.scalar.activation(out=bf, in_=bps[:128, 0:1], func=AF.Sqrt)
            al = newt.tile([128, 1], F32, tag="al")
            nc.vector.reciprocal(out=al, in_=bf)
            x_sb = newt.tile([128, 128], F32, tag="xns")
            nc.vector.tensor_scalar_mul(x_sb, ident_sb, al)
            for it in range(NEWTON_ITERS):
                yps = pacc.tile([128, 512], F32, tag="pacc")
                nc.tensor.matmul(yps[:128, :128], lhsT=g, rhs=x_sb,
                                 start=True, stop=True)
                z_sb = newt.tile([128, 128], F32, tag="zns")
                nc.vector.scalar_tensor_tensor(
                    out=z_sb, in0=ident_sb, scalar=2.0,
                    in1=yps[:128, :128], op0=ALU.mult, op1=ALU.subtract)
                xps = pacc.tile([128, 512], F32, tag="pacc")
                nc.tensor.matmul(xps[:128, :128], lhsT=x_sb, rhs=z_sb,
                                 start=True, stop=True)
                x_new = newt.tile([128, 128], F32, tag="xns")
                nc.scalar.copy(out=x_new, in_=xps[:128, :128])
                x_sb = x_new
            mps = pacc.tile([128, 512], F32, tag="pacc")
            nc.tensor.matmul(mps[:128, :128], lhsT=x_sb, rhs=x_sb,
                             start=True, stop=True)
            m2t = sup.tile([128, 128], F32, tag=f"m2_{spr}", name=f"m2_{spr}")
            nc.vector.tensor_copy(out=m2t.bitcast(F32R), in_=mps[:128, :128])
            m2_sb[spr] = m2t
            rps = pacc.tile([128, 512], F32, tag="pacc")
            nc.tensor.matmul(rps[:128, 0:1], lhsT=x_sb, rhs=ones_sb,
                             start=True, stop=True)
            rt_ = sup.tile([128, 1], F32, tag=f"r_{spr}", name=f"r_{spr}")
            nc.vector.tensor_copy(out=rt_, in_=rps[:128, 0:1])
            r_sb[spr] = rt_

        # r4[m][spr] = mask8[m, spr] * r_super(spr)
        r4 = []
        for m in range(4):
            t = smalls.tile([128, 2, 4], F32, tag="r4", name=f"r4_{m}")
            nc.vector.tensor_scalar_mul(t[:, 0, :].bitcast(F32R),
                                        mask8_sb[:, m, 0, :], r_sb[0])
            nc.vector.tensor_scalar_mul(t[:, 1, :].bitcast(F32R),
                                        mask8_sb[:, m, 1, :], r_sb[1])
            r4.append(t)

        # ================= phase 3: variance readout =================
        for m, (c, sp) in enumerate(MEMBERS):
            for ni, (no, nsz) in enumerate(_chunks(c, 512)):
                g0 = OFFS[m] + no
                psbs = []
                for spr in range(2):
                    dfp = pacc.tile([128, 512], F32, tag="pacc")
                    nc.tensor.matmul(
                        dfp[:, :nsz], lhsT=m2_sb[spr].bitcast(F32R),
                        rhs=ct_all[spr][:, g0:g0 + nsz].bitcast(F32R),
                        start=True, stop=True)
                    psb = psbp.tile([128, 512], F32, tag="psb")
                    nc.vector.tensor_mul(psb[:, :nsz].bitcast(F32R),
                                         ct_all[spr][:, g0:g0 + nsz],
                                         dfp[:, :nsz])
                    psbs.append(psb)
                qps = hq.tile([4, 512], F32, tag="hq3")
                tps = hq.tile([4, 512], F32, tag="hq3")
                for spr in range(2):
                    nc.tensor.matmul(
                        qps[:, :nsz],
                        lhsT=oq8_sb[:, m, spr, :].bitcast(F32R),
                        rhs=psbs[spr][:, :nsz].bitcast(F32R),
                        start=(spr == 0), stop=(spr == 1))
                    nc.tensor.matmul(
                        tps[:, :nsz], lhsT=r4[m][:, spr, :].bitcast(F32R),
                        rhs=ct_all[spr][:, g0:g0 + nsz].bitcast(F32R),
                        start=(spr == 0), stop=(spr == 1))
                tsq = tsqp.tile([S, 512], F32, tag="tsq")
                nc.scalar.square(out=tsq[:, :nsz], in_=tps[:S, :nsz])
                ot = outp.tile([S, 512], F32, tag="out")
                nc.vector.scalar_tensor_tensor(
                    out=ot[:, :nsz],
                    in0=tsq[:, :nsz], scalar=-1.0 / ((Q - 1) * Q),
                    in1=qps[:S, :nsz], op0=ALU.mult, op1=ALU.add)
                _dma(out=outd[:, g0:g0 + nsz], in_=ot[:, :nsz])

    nc.finalize()
    return nc


def _in_maps(xs, ws):
    ident_np, p16_np, mask8_np, k3_np = _consts()
    in_maps = []
    for i in range(NCORES):
        im = {"ident": ident_np, "p16": p16_np, "mask8": mask8_np,
              "k3": k3_np, "onesd": np.ones((128, Q), np.float32)}
        for m, (c, sp) in enumerate(MEMBERS):
            im[f"x{m}"] = np.ascontiguousarray(
                xs[m][S * i:S * (i + 1)].reshape(S, c, sp * sp), np.float32)
            im[f"wt{m}"] = np.ascontiguousarray(ws[m].T, np.float32)
        in_maps.append(im)
    return in_maps


_CACHE = {}


def kernel(x0, x1, x2, x3, W0, W1, W2, W3):
    if "nc" not in _CACHE:
        _CACHE["nc"] = _build_program()
    nc = _CACHE["nc"]
    xs = [np.asarray(x) for x in (x0, x1, x2, x3)]
    ws = [np.asarray(w) for w in (W0, W1, W2, W3)]
    res = run_bass_kernel_spmd(nc, _in_maps(xs, ws), list(range(NCORES)))
    return np.concatenate([r["out"] for r in res.results], axis=0)

